# revision 52
# baseline (speedup 1.0000x reference)
"""Trainium2 Bass kernel for nn_MegaCartTensorOut (8-core data-parallel).

Math (validated vs reference in fp64 numpy, rel err ~4e-7):
  - SelfMixTP per l: y_l = (x_l @ W_l)/sqrt(mul_l); rms_l over (32*(2l+1)).
  - (1,1,1) and (2,2,1) instructions vanish identically (antisymmetric CG
    contracted with a symmetric uu product), so the l=1 output is zero.
  - (0,2,2) and (2,0,2) are the same diagonal map; their weights combine.
  - All path/alpha/p coefficients and 1/(rms*rms) pair factors fold into the
    per-node tensor-product weights; per-(a,b,c) CG coefficients fold into
    the final per-channel contraction matmul.
  - rms_l^2 = mean+eps, so the l==l pair factors 1/(rms_l^2) need no sqrt:
    one reciprocal per l. Only the cross term 1/(rms0*rms2) takes a sqrt.

Device layout: [feature, node]. Per core 6400 node columns, processed as 4
macro-tiles of 1600 nodes = 4 groups x 400 columns packed on partitions
(128 = 4 groups x 32 channels) so DVE runs at full width.

Performance notes (vs the fp32 v1 at 234us; this version ~109us):
  - fp16 data path end to end: DVE 2x_1p mode, 1cyc/col matmuls, half DMA.
  - reciprocal_approx_fast on a [96,TN] tile replaces the 7.6us/tile DVE
    reciprocal; only the l0*l2 cross term needs one small ACT sqrt.
  - silu hoisted to a prologue: the scalar engine loads activation tables
    only twice total (silu table, then sqrt table).
  - all elementwise work on DVE: Pool/GpSimd tensor ops starve concurrent
    DVE ops of SBUF bandwidth (~10x), so Pool only issues const DMAs.
  - comps contraction software-pipelined one macro tile late so the PE
    never stalls on the DVE-late F streams; the last tile's contraction is
    split around its final F products to shorten the drain tail.
  - F0/F1/F2 share the c0 coefficient row and are pre-summed on DVE,
    saving 2 of 28 contraction matmuls.
  - nodes are ones-padded on host so the norm factors stay O(1) and fit
    fp16 without clamping.
Assumes b2 == 0 (spec fill, guaranteed by setup_inputs).
"""

import sys

sys.path.insert(0, "/opt/trn_rl_repo")

import numpy as np
from math import factorial, sqrt

N_FULL = 50000
NCORES = 8
NSHARD = 6250          # nodes per core before padding
NP = 6400              # padded nodes per core
TN = 400               # node columns per group-tile
NGROUP = 4             # node groups packed on partitions
MACRO = NP // (TN * NGROUP)   # 4 macro tiles per core
HC = 32

# ---------------- real Clebsch-Gordan (copied from the reference math) ----
def _cg(l1, l2, l3):
    f = lambda n: float(factorial(n))
    C = np.zeros((2 * l1 + 1, 2 * l2 + 1, 2 * l3 + 1))
    for m1 in range(-l1, l1 + 1):
        for m2 in range(-l2, l2 + 1):
            m3 = m1 + m2
            if abs(m3) > l3:
                continue
            pre = sqrt((2 * l3 + 1) * f(l1 + l2 - l3) * f(l1 - l2 + l3)
                       * f(-l1 + l2 + l3) / f(l1 + l2 + l3 + 1))
            pre *= sqrt(f(l3 + m3) * f(l3 - m3) * f(l1 - m1) * f(l1 + m1)
                        * f(l2 - m2) * f(l2 + m2))
            s = 0.0
            for k in range(0, l1 + l2 - l3 + 1):
                d = [k, l1 + l2 - l3 - k, l1 - m1 - k, l2 + m2 - k,
                     l3 - l2 + m1 + k, l3 - l1 - m2 + k]
                if any(x < 0 for x in d):
                    continue
                s += (-1) ** k / np.prod([f(x) for x in d])
            C[m1 + l1, m2 + l2, m3 + l3] = pre * s
    return C


def _u_real(l):
    U = np.zeros((2 * l + 1, 2 * l + 1), dtype=complex)
    U[l, l] = 1.0
    for m in range(1, l + 1):
        U[l + m, l + m] = (-1) ** m / sqrt(2)
        U[l + m, l - m] = 1.0 / sqrt(2)
        U[l - m, l + m] = -1j * (-1) ** m / sqrt(2)
        U[l - m, l - m] = 1j / sqrt(2)
    return U


def _real_cg(l1, l2, l3):
    C = _cg(l1, l2, l3).astype(complex)
    R = np.einsum("am,bn,co,mno->abc", _u_real(l1), _u_real(l2),
                  np.conj(_u_real(l3)), C)
    Rr = R.real if np.abs(R.real).max() >= np.abs(R.imag).max() else R.imag
    return (Rr / np.linalg.norm(Rr)).astype(np.float64)


_R110 = _real_cg(1, 1, 0)     # -delta/sqrt(3): sign matters
_R112 = _real_cg(1, 1, 2)
_R222 = _real_cg(2, 2, 2)
_QB = {l: _real_cg(1, 1, l) * sqrt(2 * l + 1) for l in (0, 1, 2)}
_SGN110 = float(np.sign(_R110[0, 0, 0]))   # -1

# F-stream pair lists (by-b grouping; R222 pair (0,4) is structurally zero)
_P7 = [(0, 0), (0, 1), (1, 1), (0, 2), (1, 2), (2, 2)]
_P8 = [(0, 0), (0, 1), (1, 1), (0, 2), (1, 2), (2, 2),
       (0, 3), (1, 3), (2, 3), (3, 3), (1, 4), (2, 4), (3, 4), (4, 4)]
NF = 3 + 5 + len(_P7) + len(_P8)   # 28 F streams


def _coef_tables():
    """[NF, 6] per-stream output coefficients (c0 = sph0, c1..5 = sph2)."""
    co = np.zeros((NF, 6))
    co[0, 0] = 1.0
    co[1, 0] = 1.0
    co[2, 0] = 1.0
    for cc in range(5):
        co[3 + cc, 1 + cc] = 1.0
    for k, (a, b) in enumerate(_P7):
        co[8 + k, 1:] = _R112[a, b, :] * (2.0 if a < b else 1.0)
    for k, (a, b) in enumerate(_P8):
        co[14 + k, 1:] = _R222[a, b, :] * (2.0 if a < b else 1.0)
    return co


_COEF6 = _coef_tables()

_NC_CACHE = {}


def _build_nc():
    import concourse.bacc as bacc
    import concourse.mybir as mybir
    import concourse.tile as tile

    f32 = mybir.dt.float32
    f16 = mybir.dt.float16
    AF = mybir.ActivationFunctionType

    nc = bacc.Bacc("TRN2", target_bir_lowering=False, debug=False)

    XS = nc.declare_dram_parameter("xs", [128, NP], f16, isOutput=False)
    XB = nc.declare_dram_parameter("xb", [MACRO, 128, 6000], f16,
                                   isOutput=False)
    A1d = nc.declare_dram_parameter("a1", [2, 128, 128], f16, isOutput=False)
    W0d = nc.declare_dram_parameter("w0", [4, 128, 128], f16, isOutput=False)
    W1d = nc.declare_dram_parameter("w1", [2, 128, 128], f16, isOutput=False)
    W2d = nc.declare_dram_parameter("w2", [128, 128], f16, isOutput=False)
    A2d = nc.declare_dram_parameter("a2", [12, 128, 128], f16, isOutput=False)
    ONd = nc.declare_dram_parameter("on96", [128, 96], f16, isOutput=False)
    PXd = nc.declare_dram_parameter("pbx", [96, 128], f16, isOutput=False)
    COd = nc.declare_dram_parameter("coef", [26, 128, 24], f16, isOutput=False)
    B1d = nc.declare_dram_parameter("b1r", [128, 1], f32, isOutput=False)
    OUT = nc.declare_dram_parameter("out", [MACRO, 24, TN], f32,
                                    isOutput=True)

    X0OFF, X1OFF, X2OFF = 0, 1600, 4000

    with tile.TileContext(nc) as tc:
        with tc.tile_pool(name="const", bufs=1) as cp, \
             tc.tile_pool(name="dmain", bufs=4) as dp, \
             tc.tile_pool(name="work", bufs=2) as wp, \
             tc.tile_pool(name="psum", bufs=1, space="PSUM") as pp:

            # ---- constants + inputs; issue order favors the critical path
            # (xs/a1/b1r for phase A, xb[0] + mix weights for tile 0), and
            # issues are spread across engine DMA queues so they don't
            # serialize on the sync sequencer at ~0.8us each.
            xs = cp.tile([128, NP], f16)
            nc.sync.dma_start(xs[:, 0:NGROUP * TN], XS[:, 0:NGROUP * TN])
            a1 = cp.tile([128, 2 * 128], f16)
            nc.sync.dma_start(a1[:].rearrange("p (q m) -> p q m", q=2),
                              A1d[:].rearrange("q p m -> p q m"))
            b1r = cp.tile([128, 1], f32)
            nc.sync.dma_start(b1r[:], B1d[:])
            xb_t = []
            for t in range(MACRO):
                xb_t.append(dp.tile([128, 6000], f16, tag="xb",
                                    name=f"xb{t}"))
            nc.sync.dma_start(xb_t[0][:, 0:1600], XB[0][:, 0:1600])
            nc.sync.dma_start(xb_t[0][:, 1600:4000], XB[0][:, 1600:4000])
            nc.sync.dma_start(xb_t[0][:, 4000:6000], XB[0][:, 4000:6000])
            for t in range(1, MACRO):
                c0 = t * NGROUP * TN
                nc.sync.dma_start(xs[:, c0:c0 + NGROUP * TN],
                                  XS[:, c0:c0 + NGROUP * TN])
            w0 = cp.tile([128, 4 * 128], f16)
            nc.gpsimd.dma_start(w0[:].rearrange("p (g m) -> p g m", g=4),
                                W0d[:].rearrange("g p m -> p g m"))
            w1 = cp.tile([128, 2 * 128], f16)
            nc.gpsimd.dma_start(w1[:].rearrange("p (q m) -> p q m", q=2),
                                W1d[:].rearrange("q p m -> p q m"))
            w2 = cp.tile([128, 128], f16)
            nc.gpsimd.dma_start(w2[:], W2d[:])
            on96 = cp.tile([128, 96], f16)
            nc.gpsimd.dma_start(on96[:], ONd[:])
            pbx = cp.tile([96, 128], f16)
            nc.gpsimd.dma_start(pbx[:], PXd[:])
            a2 = cp.tile([128, 12 * 128], f16)
            nc.gpsimd.dma_start(a2[:].rearrange("p (j m) -> p j m", j=12),
                                A2d[:].rearrange("j p m -> p j m"))
            co = cp.tile([128, 26 * 24], f16)
            nc.gpsimd.dma_start(co[:].rearrange("p (k m) -> p k m", k=26),
                                COd[:].rearrange("k p m -> p k m"))
            for t in range(1, MACRO):
                nc.sync.dma_start(xb_t[t][:], XB[t])

            def emit_mix(t):
                # mix: y tiles packed (group, chan), evac to ystack
                xb = xb_t[t]
                ystack = wp.tile([128, 9 * TN], f16, tag="ystack",
                                 name=f"ystack{t}")
                y0ps = pp.tile([128, TN], f32, tag="yps", bufs=2)
                for g in range(4):
                    nc.tensor.matmul(y0ps[:], w0[:, g * 128:(g + 1) * 128],
                                     xb[:, X0OFF + g * TN:X0OFF + (g + 1) * TN],
                                     start=(g == 0), stop=(g == 3))
                nc.scalar.copy(ystack[:, 0:TN], y0ps[:])
                for m in range(3):
                    yps = pp.tile([128, TN], f32, tag="yps", bufs=2)
                    for p in range(2):
                        o = X1OFF + m * 2 * TN + p * TN
                        nc.tensor.matmul(yps[:], w1[:, p * 128:(p + 1) * 128],
                                         xb[:, o:o + TN],
                                         start=(p == 0), stop=(p == 1))
                    nc.scalar.copy(ystack[:, (1 + m) * TN:(2 + m) * TN], yps[:])
                for m in range(5):
                    yps = pp.tile([128, TN], f32, tag="yps", bufs=2)
                    o = X2OFF + m * TN
                    nc.tensor.matmul(yps[:], w2[:], xb[:, o:o + TN],
                                     start=True, stop=True)
                    nc.scalar.copy(ystack[:, (4 + m) * TN:(5 + m) * TN], yps[:])
                return ystack

            # tile-0 mix first so the PE has work as soon as the first
            # input block lands (phase A's silu chain would stall it)
            ystack0 = emit_mix(0)

            # ---- phase A: h = silu(x_scalar @ A1 + b1) for all tiles ------
            # (keeps the scalar engine on the silu table before the single
            #  switch to the sqrt table for the rest of the kernel)
            hsb_t = []
            for t in range(MACRO):
                hh = cp.tile([128, 2 * TN], f16, tag=f"hsb{t}")
                for p in range(2):
                    hps = pp.tile([128, TN], f32, tag="acc")
                    for q in range(2):
                        g = 2 * p + q
                        c0 = t * NGROUP * TN + g * TN
                        nc.tensor.matmul(hps[:], a1[:, q * 128:(q + 1) * 128],
                                         xs[:, c0:c0 + TN],
                                         start=(q == 0), stop=(q == 1))
                    nc.scalar.activation(hh[:, p * TN:(p + 1) * TN], hps[:],
                                         AF.Silu, bias=b1r[:, 0:1])
                hsb_t.append(hh)

            def emit_comps(fsb, fsb2, t):
                # contraction over channels with CG coefficients; software
                # pipelined one iteration late so the PE never waits on the
                # (DVE-late) F streams
                comps = pp.tile([24, TN], f32, tag="acc")
                for k in range(26):
                    mv = (fsb[:, k * TN:(k + 1) * TN] if k < 18 else
                          fsb2[:, (k - 18) * TN:(k - 17) * TN])
                    nc.tensor.matmul(comps[:], co[:, k * 24:(k + 1) * 24],
                                     mv, start=(k == 0), stop=(k == 25))
                csb = wp.tile([24, TN], f32, tag="csb")
                nc.scalar.copy(csb[:], comps[:])
                nc.sync.dma_start(OUT[t], csb[:])

            prev = None
            for t in range(MACRO):
                ystack = ystack0 if t == 0 else emit_mix(t)

                if prev is not None:
                    emit_comps(*prev)

                # ---- squares + per-l sums (DVE; Pool sharing a tile with
                # DVE stalls DVE reads ~10x, so Pool gets isolated work) ----
                sq = wp.tile([128, 9 * TN], f16, tag="sq")
                nc.vector.tensor_mul(sq[:], ystack[:], ystack[:])
                ssq = wp.tile([128, 2 * TN], f16, tag="ssq")
                tmp2 = wp.tile([128, 2 * TN], f16, tag="tmp2")
                # (y1[0]^2+y1[1]^2 | y2[0]^2+y2[1]^2)
                ia = sq[:, TN:9 * TN].rearrange("p (k n) -> p k n", k=8)
                nc.vector.tensor_add(
                    tmp2[:].rearrange("p (k n) -> p k n", k=2),
                    ia[:, 0:4:3, :], ia[:, 1:5:3, :])
                nc.vector.tensor_add(ssq[:, 0:TN], tmp2[:, 0:TN],
                                     sq[:, 3 * TN:4 * TN])
                t2 = wp.tile([128, TN], f16, tag="t2")
                nc.vector.tensor_add(t2[:], tmp2[:, TN:2 * TN],
                                     sq[:, 6 * TN:7 * TN])
                nc.vector.tensor_add(t2[:], t2[:], sq[:, 7 * TN:8 * TN])
                nc.vector.tensor_add(ssq[:, TN:2 * TN], t2[:],
                                     sq[:, 8 * TN:9 * TN])

                # ---- norm factors: pat_l = 1/(mean_l + eps), no sqrt ------
                rsum = pp.tile([96, TN], f32, tag="rb", bufs=2)
                for l, rhs in enumerate((sq[:, 0:TN], ssq[:, 0:TN],
                                         ssq[:, TN:2 * TN])):
                    nc.tensor.matmul(rsum[32 * l:32 * l + 32, :],
                                     on96[:, 32 * l:32 * l + 32], rhs,
                                     start=True, stop=True)
                msq = wp.tile([96, TN], f32, tag="msq")
                nc.vector.tensor_scalar_add(msq[:], rsum[:], 1e-5)
                pat = wp.tile([96, TN], f32, tag="pat")
                nc.vector.reciprocal_approx_fast(pat[:], msq[:])
                # nodes are ones-padded on host, so pat is O(1) everywhere
                pat16 = wp.tile([96, TN], f16, tag="pat16")
                nc.vector.tensor_copy(pat16[:], pat[:])

                # broadcast patterns to (group, chan) partitions; cross term
                # 1/(rms0*rms2) = sqrt(pat0*pat2) computed post-broadcast
                bsb = wp.tile([128, 4 * TN], f16, tag="bsb")
                for l in range(3):
                    bps = pp.tile([128, TN], f32, tag="rb", bufs=2)
                    nc.tensor.matmul(bps[:], pbx[32 * l:32 * l + 4, :],
                                     pat16[32 * l:32 * l + 4, :],
                                     start=True, stop=True)
                    nc.scalar.copy(bsb[:, l * TN:(l + 1) * TN], bps[:])
                sqf3 = wp.tile([128, TN], f16, tag="sqf3")
                nc.vector.tensor_mul(sqf3[:], bsb[:, 0:TN],
                                     bsb[:, 2 * TN:3 * TN])
                nc.scalar.activation(bsb[:, 3 * TN:4 * TN], sqf3[:], AF.Sqrt)

                # ---- fold rms pairs into tp weights -----------------------
                # wsb order: g0 g1 g2 g56 g7 g8 ; pattern j -> 0 1 2 3 1 2
                wq = wp.tile([128, 6 * TN], f16, tag="wq")
                for j in range(6):
                    wps = pp.tile([128, TN], f32, tag="wps", bufs=2)
                    for pr in range(2):
                        nc.tensor.matmul(wps[:],
                                         a2[:, (2 * j + pr) * 128:
                                              (2 * j + pr + 1) * 128],
                                         hsb_t[t][:, pr * TN:(pr + 1) * TN],
                                         start=(pr == 0), stop=(pr == 1))
                    nc.scalar.copy(wq[:, j * TN:(j + 1) * TN], wps[:])
                wsb = wp.tile([128, 6 * TN], f16, tag="wsb")
                nc.vector.tensor_mul(wsb[:, 0:4 * TN], wq[:, 0:4 * TN],
                                     bsb[:, 0:4 * TN])
                nc.vector.tensor_mul(wsb[:, 4 * TN:6 * TN],
                                     wq[:, 4 * TN:6 * TN],
                                     bsb[:, TN:3 * TN])

                # ---- TP products into F streams ---------------------------
                # F0+F1+F2 share the c0 coef row, so they merge into ONE
                # contraction stream (saves 2 comps matmuls per tile)
                fsb = wp.tile([128, 18 * TN], f16, tag="fsb")
                f12 = wp.tile([128, 2 * TN], f16, tag="f12")
                nc.vector.tensor_mul(fsb[:, 0:TN], wsb[:, 0:TN], sq[:, 0:TN])
                nc.vector.tensor_mul(f12[:], wsb[:, TN:3 * TN],
                                     ssq[:, 0:2 * TN])
                nc.vector.tensor_add(fsb[:, 0:TN], fsb[:, 0:TN], f12[:, 0:TN])
                nc.vector.tensor_add(fsb[:, 0:TN], fsb[:, 0:TN],
                                     f12[:, TN:2 * TN])
                # i56: wy0 = g56'*y0 ; F[c] = wy0*y2[c]
                wy0 = wp.tile([128, TN], f16, tag="wy0")
                nc.vector.tensor_mul(wy0[:], wsb[:, 3 * TN:4 * TN],
                                     ystack[:, 0:TN])
                nc.vector.tensor_mul(
                    fsb[:, TN:6 * TN].rearrange("p (k n) -> p k n", k=5),
                    wy0[:].unsqueeze(1).broadcast_to((128, 5, TN)),
                    ystack[:, 4 * TN:9 * TN].rearrange("p (k n) -> p k n", k=5))
                # i7: wy1[a] = g7'*y1[a] ; F pairs by b
                wy1 = wp.tile([128, 3 * TN], f16, tag="wy1")
                nc.vector.tensor_mul(
                    wy1[:].rearrange("p (k n) -> p k n", k=3),
                    wsb[:, 4 * TN:5 * TN].unsqueeze(1).broadcast_to((128, 3, TN)),
                    ystack[:, TN:4 * TN].rearrange("p (k n) -> p k n", k=3))
                off = 6 * TN
                for b in range(3):
                    w_ = (b + 1)
                    nc.vector.tensor_mul(
                        fsb[:, off:off + w_ * TN].rearrange(
                            "p (k n) -> p k n", k=w_),
                        wy1[:, 0:w_ * TN].rearrange("p (k n) -> p k n", k=w_),
                        ystack[:, (1 + b) * TN:(2 + b) * TN]
                        .unsqueeze(1).broadcast_to((128, w_, TN)))
                    off += w_ * TN
                # i8: wy2[a] = g8'*y2[a] ; F pairs by b (skip (0,4));
                # b=0..2 on DVE into fsb, b=3..4 on Pool into its own tile
                wy2 = wp.tile([128, 5 * TN], f16, tag="wy2")
                nc.vector.tensor_mul(
                    wy2[:].rearrange("p (k n) -> p k n", k=5),
                    wsb[:, 5 * TN:6 * TN].unsqueeze(1).broadcast_to((128, 5, TN)),
                    ystack[:, 4 * TN:9 * TN].rearrange("p (k n) -> p k n", k=5))
                for b in range(3):
                    w_ = b + 1
                    nc.vector.tensor_mul(
                        fsb[:, off:off + w_ * TN].rearrange(
                            "p (k n) -> p k n", k=w_),
                        wy2[:, 0:w_ * TN].rearrange("p (k n) -> p k n", k=w_),
                        ystack[:, (4 + b) * TN:(5 + b) * TN]
                        .unsqueeze(1).broadcast_to((128, w_, TN)))
                    off += w_ * TN
                # last tile: its comps can't hide behind a next tile, so
                # start the contraction on the already-complete streams
                # before the final fsb2 products are issued
                last = (t == MACRO - 1)
                if last:
                    comps = pp.tile([24, TN], f32, tag="acc")
                    for k in range(18):
                        nc.tensor.matmul(comps[:], co[:, k * 24:(k + 1) * 24],
                                         fsb[:, k * TN:(k + 1) * TN],
                                         start=(k == 0), stop=False)

                # Pool tensor ops starve concurrent DVE ops of SBUF
                # bandwidth (~10x slowdown), so everything stays on DVE
                fsb2 = wp.tile([128, 8 * TN], f16, tag="fsb2")
                nc.vector.tensor_mul(
                    fsb2[:, 0:4 * TN].rearrange("p (k n) -> p k n", k=4),
                    wy2[:, 0:4 * TN].rearrange("p (k n) -> p k n", k=4),
                    ystack[:, 7 * TN:8 * TN]
                    .unsqueeze(1).broadcast_to((128, 4, TN)))
                nc.vector.tensor_mul(
                    fsb2[:, 4 * TN:8 * TN].rearrange("p (k n) -> p k n", k=4),
                    wy2[:, TN:5 * TN].rearrange("p (k n) -> p k n", k=4),
                    ystack[:, 8 * TN:9 * TN]
                    .unsqueeze(1).broadcast_to((128, 4, TN)))

                if last:
                    for k in range(18, 26):
                        nc.tensor.matmul(comps[:], co[:, k * 24:(k + 1) * 24],
                                         fsb2[:, (k - 18) * TN:(k - 17) * TN],
                                         start=False, stop=(k == 25))
                    csb = wp.tile([24, TN], f32, tag="csb")
                    nc.scalar.copy(csb[:], comps[:])
                    nc.sync.dma_start(OUT[t], csb[:])
                else:
                    prev = (fsb, fsb2, t)

    nc.compile()
    return nc


def _host_prep(inputs):
    xs = np.asarray(inputs["x_scalar"], dtype=np.float32)
    xq = np.asarray(inputs["x_spherical"], dtype=np.float32)
    W0 = np.asarray(inputs["W0"], np.float32)
    W1 = np.asarray(inputs["W1"], np.float32)
    W2 = np.asarray(inputs["W2"], np.float32)
    A1 = np.asarray(inputs["A1"], np.float32)
    b1 = np.asarray(inputs["b1"], np.float32)
    A2 = np.asarray(inputs["A2"], np.float32)
    p0 = np.asarray(inputs["p0"], np.float64)
    p2 = np.asarray(inputs["p2"], np.float64)

    NPAD = NCORES * NP
    xsp = np.ones((NPAD, 128), np.float32)
    xqp = np.ones((NPAD, 480), np.float32)
    for i in range(NCORES):
        s = slice(i * NSHARD, (i + 1) * NSHARD)
        d = slice(i * NP, i * NP + NSHARD)
        xsp[d] = xs[s]
        xqp[d] = xq[s]

    # per-core transposed shards (fp16, one packed tensor per macro tile)
    shards = []
    for i in range(NCORES):
        blk = xqp[i * NP:(i + 1) * NP]           # [NP, 480]
        x0t = blk[:, :128].T                     # [128, NP]
        x1t = blk[:, 128:320].reshape(NP, 64, 3).transpose(2, 1, 0)
        v1 = x1t.reshape(3, 64, MACRO, 2, 2, TN)        # m u t p q n
        x1t = v1.transpose(0, 2, 4, 1, 3, 5).reshape(3, MACRO, 128, 2 * TN)
        x2t = blk[:, 320:480].reshape(NP, 32, 5).transpose(2, 1, 0)
        v2 = x2t.reshape(5, 32, MACRO, 4, TN)           # m u t g n
        x2t = v2.transpose(0, 2, 3, 1, 4).reshape(5, MACRO, 128, TN)
        # xb[t] = [x0 (1600) | x1 m-major (2400) | x2 m-major (2000)]
        xb = np.empty((MACRO, 128, 6000), np.float16)
        for t in range(MACRO):
            xb[t, :, 0:1600] = x0t[:, t * 1600:(t + 1) * 1600]
            xb[t, :, 1600:4000] = (x1t[:, t].transpose(1, 0, 2)
                                   .reshape(128, 2400))
            xb[t, :, 4000:6000] = (x2t[:, t].transpose(1, 0, 2)
                                   .reshape(128, 2000))
        xst = np.ascontiguousarray(
            xsp[i * NP:(i + 1) * NP].T).astype(np.float16)
        shards.append((xst, np.ascontiguousarray(xb)))

    # folded constants
    alpha0 = 1.0 / sqrt(3 * HC)
    alpha2 = sqrt(5.0) / sqrt(4 * HC)
    cJ = [alpha0 * p0[0], _SGN110 * alpha0 * p0[1] / sqrt(3),
          alpha0 * p0[2] / sqrt(5)]
    cJ = [c / sqrt(3) for c in cJ]
    a2f = np.zeros((6, 64, 32), np.float64)
    a2f[0] = A2[:, 0:32] * cJ[0]
    a2f[1] = A2[:, 32:64] * cJ[1]
    a2f[2] = A2[:, 64:96] * cJ[2]
    a2f[3] = (alpha2 / (2 * sqrt(5))) * (p2[0] * A2[:, 160:192]
                                         + p2[1] * A2[:, 192:224])
    a2f[4] = A2[:, 224:256] * (alpha2 * p2[2] / 2.0)
    a2f[5] = A2[:, 256:288] * (alpha2 * p2[3] / 2.0)
    a2bd = np.zeros((6, 2, 128, 128), np.float32)
    for j in range(6):
        for pr in range(2):
            for q in range(2):
                g = 2 * pr + q
                a2bd[j, pr, 64 * q:64 * (q + 1), 32 * g:32 * (g + 1)] = a2f[j]
    a2bd = a2bd.reshape(12, 128, 128)

    w1bd = np.zeros((2, 128, 128), np.float32)
    for p in range(2):
        for q in range(2):
            g = 2 * p + q
            w1bd[p, 64 * q:64 * (q + 1), 32 * g:32 * (g + 1)] = W1 / sqrt(64)
    w2bd = np.zeros((128, 128), np.float32)
    for g in range(4):
        w2bd[32 * g:32 * (g + 1), 32 * g:32 * (g + 1)] = W2 / sqrt(32)

    a1bd = np.zeros((2, 128, 128), np.float32)
    for q in range(2):
        a1bd[q, :, 64 * q:64 * (q + 1)] = A1
    w0bd = np.zeros((4, 128, 128), np.float32)
    for g in range(4):
        w0bd[g, :, 32 * g:32 * (g + 1)] = W0 / sqrt(128)

    # [128, 96] selector: col 32l+g contracts group g scaled by 1/(HC(2l+1))
    on96 = np.zeros((128, 96), np.float32)
    for l in range(3):
        for g in range(4):
            on96[32 * g:32 * (g + 1), 32 * l + g] = 1.0 / (HC * (2 * l + 1))

    # [96, 128] broadcast selector, replicated at each 32l block so the
    # stationary/moving base partitions match (row 32l+g -> group g chans)
    pbx = np.zeros((96, 128), np.float32)
    for l in range(3):
        for g in range(4):
            pbx[32 * l + g, 32 * g:32 * (g + 1)] = 1.0
    c26 = np.concatenate([_COEF6[0:1], _COEF6[3:]], axis=0)   # F012 merged
    coef = np.zeros((26, 128, 24), np.float32)
    for k in range(26):
        for g in range(4):
            coef[k, 32 * g:32 * (g + 1), 6 * g:6 * (g + 1)] = c26[k]

    const = {
        "a1": a1bd.astype(np.float16),
        "w0": w0bd.astype(np.float16),
        "w1": w1bd.astype(np.float16),
        "w2": w2bd.astype(np.float16),
        "a2": a2bd.astype(np.float16),
        "on96": on96.astype(np.float16),
        "pbx": pbx.astype(np.float16),
        "coef": coef.astype(np.float16),
        "b1r": np.concatenate([b1, b1]).reshape(128, 1).astype(np.float32),
    }
    return shards, const


def kernel(**inputs):
    from concourse.bass_utils import run_bass_kernel_spmd

    if "nc" not in _NC_CACHE:
        _NC_CACHE["nc"] = _build_nc()
    nc = _NC_CACHE["nc"]

    shards, const = _host_prep(inputs)
    in_maps = []
    for i in range(NCORES):
        xst, xbt = shards[i]
        m = {"xs": xst, "xb": xbt}
        m.update(const)
        in_maps.append(m)

    res = run_bass_kernel_spmd(nc, in_maps, list(range(NCORES)))
    snode = np.concatenate(
        [res.results[i]["out"].reshape(MACRO, 4, 6, TN)
         .transpose(2, 0, 1, 3).reshape(6, NP)[:, :NSHARD]
         for i in range(NCORES)], axis=1)

    # sph (6 comps) -> cartesian 3x3, segment-sum, roll
    Q6 = np.concatenate([_QB[0].reshape(9, 1), _QB[2].reshape(9, 5)],
                        axis=1).astype(np.float32)     # [9, 6]
    cart = snode.T @ Q6.T                              # [N, 9]
    batch = np.asarray(inputs["batch"])
    B = int(inputs["num_graphs"])
    idx = np.searchsorted(batch, np.arange(B))
    g = np.add.reduceat(cart, idx, axis=0)
    g[np.diff(np.concatenate([idx, [N_FULL]])) == 0] = 0
    out = g.reshape(B, 3, 3).astype(np.float32)
    return np.roll(np.roll(out, 1, axis=1), 1, axis=2)


# revision 53
# speedup vs baseline: 1.0397x; 1.0397x over previous
"""Trainium2 Bass kernel for nn_MegaCartTensorOut (8-core data-parallel).

Math (validated vs reference in fp64 numpy, rel err ~4e-7):
  - SelfMixTP per l: y_l = (x_l @ W_l)/sqrt(mul_l); rms_l over (32*(2l+1)).
  - (1,1,1) and (2,2,1) instructions vanish identically (antisymmetric CG
    contracted with a symmetric uu product), so the l=1 output is zero.
  - (0,2,2) and (2,0,2) are the same diagonal map; their weights combine.
  - All path/alpha/p coefficients and 1/(rms*rms) pair factors fold into the
    per-node tensor-product weights; per-(a,b,c) CG coefficients fold into
    the final per-channel contraction matmul.
  - rms_l^2 = mean+eps, so the l==l pair factors 1/(rms_l^2) need no sqrt:
    one reciprocal per l. Only the cross term 1/(rms0*rms2) takes a sqrt.

Device layout: [feature, node]. Per core 6400 node columns, processed as 4
macro-tiles of 1600 nodes = 4 groups x 400 columns packed on partitions
(128 = 4 groups x 32 channels) so DVE runs at full width.

Performance notes (vs the fp32 v1 at 234us; this version ~109us):
  - fp16 data path end to end: DVE 2x_1p mode, 1cyc/col matmuls, half DMA.
  - reciprocal_approx_fast on a [96,TN] tile replaces the 7.6us/tile DVE
    reciprocal; only the l0*l2 cross term needs one small ACT sqrt.
  - silu hoisted to a prologue: the scalar engine loads activation tables
    only twice total (silu table, then sqrt table).
  - all elementwise work on DVE: Pool/GpSimd tensor ops starve concurrent
    DVE ops of SBUF bandwidth (~10x), so Pool only issues const DMAs.
  - comps contraction software-pipelined one macro tile late so the PE
    never stalls on the DVE-late F streams; the last tile's contraction is
    split around its final F products to shorten the drain tail.
  - F0/F1/F2 share the c0 coefficient row and are pre-summed on DVE,
    saving 2 of 28 contraction matmuls.
  - nodes are ones-padded on host so the norm factors stay O(1) and fit
    fp16 without clamping.
Assumes b2 == 0 (spec fill, guaranteed by setup_inputs).
"""

import sys

sys.path.insert(0, "/opt/trn_rl_repo")

import numpy as np
from math import factorial, sqrt

N_FULL = 50000
NCORES = 8
NSHARD = 6250          # nodes per core before padding
NP = 6400              # padded nodes per core
TN = 400               # node columns per group-tile
NGROUP = 4             # node groups packed on partitions
MACRO = NP // (TN * NGROUP)   # 4 macro tiles per core
HC = 32

# ---------------- real Clebsch-Gordan (copied from the reference math) ----
def _cg(l1, l2, l3):
    f = lambda n: float(factorial(n))
    C = np.zeros((2 * l1 + 1, 2 * l2 + 1, 2 * l3 + 1))
    for m1 in range(-l1, l1 + 1):
        for m2 in range(-l2, l2 + 1):
            m3 = m1 + m2
            if abs(m3) > l3:
                continue
            pre = sqrt((2 * l3 + 1) * f(l1 + l2 - l3) * f(l1 - l2 + l3)
                       * f(-l1 + l2 + l3) / f(l1 + l2 + l3 + 1))
            pre *= sqrt(f(l3 + m3) * f(l3 - m3) * f(l1 - m1) * f(l1 + m1)
                        * f(l2 - m2) * f(l2 + m2))
            s = 0.0
            for k in range(0, l1 + l2 - l3 + 1):
                d = [k, l1 + l2 - l3 - k, l1 - m1 - k, l2 + m2 - k,
                     l3 - l2 + m1 + k, l3 - l1 - m2 + k]
                if any(x < 0 for x in d):
                    continue
                s += (-1) ** k / np.prod([f(x) for x in d])
            C[m1 + l1, m2 + l2, m3 + l3] = pre * s
    return C


def _u_real(l):
    U = np.zeros((2 * l + 1, 2 * l + 1), dtype=complex)
    U[l, l] = 1.0
    for m in range(1, l + 1):
        U[l + m, l + m] = (-1) ** m / sqrt(2)
        U[l + m, l - m] = 1.0 / sqrt(2)
        U[l - m, l + m] = -1j * (-1) ** m / sqrt(2)
        U[l - m, l - m] = 1j / sqrt(2)
    return U


def _real_cg(l1, l2, l3):
    C = _cg(l1, l2, l3).astype(complex)
    R = np.einsum("am,bn,co,mno->abc", _u_real(l1), _u_real(l2),
                  np.conj(_u_real(l3)), C)
    Rr = R.real if np.abs(R.real).max() >= np.abs(R.imag).max() else R.imag
    return (Rr / np.linalg.norm(Rr)).astype(np.float64)


_R110 = _real_cg(1, 1, 0)     # -delta/sqrt(3): sign matters
_R112 = _real_cg(1, 1, 2)
_R222 = _real_cg(2, 2, 2)
_QB = {l: _real_cg(1, 1, l) * sqrt(2 * l + 1) for l in (0, 1, 2)}
_SGN110 = float(np.sign(_R110[0, 0, 0]))   # -1

# F-stream pair lists (by-b grouping; R222 pair (0,4) is structurally zero)
_P7 = [(0, 0), (0, 1), (1, 1), (0, 2), (1, 2), (2, 2)]
_P8 = [(0, 0), (0, 1), (1, 1), (0, 2), (1, 2), (2, 2),
       (0, 3), (1, 3), (2, 3), (3, 3), (1, 4), (2, 4), (3, 4), (4, 4)]
NF = 3 + 5 + len(_P7) + len(_P8)   # 28 F streams


def _coef_tables():
    """[NF, 6] per-stream output coefficients (c0 = sph0, c1..5 = sph2)."""
    co = np.zeros((NF, 6))
    co[0, 0] = 1.0
    co[1, 0] = 1.0
    co[2, 0] = 1.0
    for cc in range(5):
        co[3 + cc, 1 + cc] = 1.0
    for k, (a, b) in enumerate(_P7):
        co[8 + k, 1:] = _R112[a, b, :] * (2.0 if a < b else 1.0)
    for k, (a, b) in enumerate(_P8):
        co[14 + k, 1:] = _R222[a, b, :] * (2.0 if a < b else 1.0)
    return co


_COEF6 = _coef_tables()

_NC_CACHE = {}


def _build_nc():
    import concourse.bacc as bacc
    import concourse.mybir as mybir
    import concourse.tile as tile

    f32 = mybir.dt.float32
    f16 = mybir.dt.float16
    AF = mybir.ActivationFunctionType

    nc = bacc.Bacc("TRN2", target_bir_lowering=False, debug=False)

    XS = nc.declare_dram_parameter("xs", [128, NP], f16, isOutput=False)
    XB = nc.declare_dram_parameter("xb", [MACRO, 128, 6000], f16,
                                   isOutput=False)
    A1d = nc.declare_dram_parameter("a1", [2, 128, 128], f16, isOutput=False)
    W0d = nc.declare_dram_parameter("w0", [4, 128, 128], f16, isOutput=False)
    W1d = nc.declare_dram_parameter("w1", [2, 128, 128], f16, isOutput=False)
    W2d = nc.declare_dram_parameter("w2", [128, 128], f16, isOutput=False)
    A2d = nc.declare_dram_parameter("a2", [12, 128, 128], f16, isOutput=False)
    ONd = nc.declare_dram_parameter("on96", [128, 96], f16, isOutput=False)
    PXd = nc.declare_dram_parameter("pbx", [96, 128], f16, isOutput=False)
    COd = nc.declare_dram_parameter("coef", [26, 128, 24], f16, isOutput=False)
    B1d = nc.declare_dram_parameter("b1r", [128, 1], f32, isOutput=False)
    OUT = nc.declare_dram_parameter("out", [MACRO, 24, TN], f32,
                                    isOutput=True)

    X0OFF, X1OFF, X2OFF = 0, 1600, 4000

    with tile.TileContext(nc) as tc:
        with tc.tile_pool(name="const", bufs=1) as cp, \
             tc.tile_pool(name="dmain", bufs=4) as dp, \
             tc.tile_pool(name="work", bufs=2) as wp, \
             tc.tile_pool(name="psum", bufs=1, space="PSUM") as pp:

            # ---- constants + inputs; issue order favors the critical path
            # (xs/a1/b1r for phase A, xb[0] + mix weights for tile 0), and
            # issues are spread across engine DMA queues so they don't
            # serialize on the sync sequencer at ~0.8us each.
            xs = cp.tile([128, NP], f16)
            nc.sync.dma_start(xs[:, 0:NGROUP * TN], XS[:, 0:NGROUP * TN])
            a1 = cp.tile([128, 2 * 128], f16)
            nc.sync.dma_start(a1[:].rearrange("p (q m) -> p q m", q=2),
                              A1d[:].rearrange("q p m -> p q m"))
            b1r = cp.tile([128, 1], f32)
            nc.sync.dma_start(b1r[:], B1d[:])
            xb_t = []
            for t in range(MACRO):
                xb_t.append(dp.tile([128, 6000], f16, tag="xb",
                                    name=f"xb{t}"))
            nc.sync.dma_start(xb_t[0][:, 0:1600], XB[0][:, 0:1600])
            nc.sync.dma_start(xb_t[0][:, 1600:4000], XB[0][:, 1600:4000])
            nc.sync.dma_start(xb_t[0][:, 4000:6000], XB[0][:, 4000:6000])
            for t in range(1, MACRO):
                c0 = t * NGROUP * TN
                nc.sync.dma_start(xs[:, c0:c0 + NGROUP * TN],
                                  XS[:, c0:c0 + NGROUP * TN])
            w0 = cp.tile([128, 4 * 128], f16)
            nc.gpsimd.dma_start(w0[:].rearrange("p (g m) -> p g m", g=4),
                                W0d[:].rearrange("g p m -> p g m"))
            w1 = cp.tile([128, 2 * 128], f16)
            nc.gpsimd.dma_start(w1[:].rearrange("p (q m) -> p q m", q=2),
                                W1d[:].rearrange("q p m -> p q m"))
            w2 = cp.tile([128, 128], f16)
            nc.gpsimd.dma_start(w2[:], W2d[:])
            on96 = cp.tile([128, 96], f16)
            nc.gpsimd.dma_start(on96[:], ONd[:])
            pbx = cp.tile([96, 128], f16)
            nc.gpsimd.dma_start(pbx[:], PXd[:])
            a2 = cp.tile([128, 12 * 128], f16)
            nc.gpsimd.dma_start(a2[:].rearrange("p (j m) -> p j m", j=12),
                                A2d[:].rearrange("j p m -> p j m"))
            co = cp.tile([128, 26 * 24], f16)
            nc.gpsimd.dma_start(co[:].rearrange("p (k m) -> p k m", k=26),
                                COd[:].rearrange("k p m -> p k m"))
            for t in range(1, MACRO):
                nc.sync.dma_start(xb_t[t][:], XB[t])

            def emit_mix(t):
                # mix: y tiles packed (group, chan), evac to ystack
                xb = xb_t[t]
                ystack = wp.tile([128, 9 * TN], f16, tag="ystack",
                                 name=f"ystack{t}")
                y0ps = pp.tile([128, TN], f32, tag="yps", bufs=2)
                for g in range(4):
                    nc.tensor.matmul(y0ps[:], w0[:, g * 128:(g + 1) * 128],
                                     xb[:, X0OFF + g * TN:X0OFF + (g + 1) * TN],
                                     start=(g == 0), stop=(g == 3))
                nc.scalar.copy(ystack[:, 0:TN], y0ps[:])
                for m in range(3):
                    yps = pp.tile([128, TN], f32, tag="yps", bufs=2)
                    for p in range(2):
                        o = X1OFF + m * 2 * TN + p * TN
                        nc.tensor.matmul(yps[:], w1[:, p * 128:(p + 1) * 128],
                                         xb[:, o:o + TN],
                                         start=(p == 0), stop=(p == 1))
                    nc.scalar.copy(ystack[:, (1 + m) * TN:(2 + m) * TN], yps[:])
                for m in range(5):
                    yps = pp.tile([128, TN], f32, tag="yps", bufs=2)
                    o = X2OFF + m * TN
                    nc.tensor.matmul(yps[:], w2[:], xb[:, o:o + TN],
                                     start=True, stop=True)
                    nc.scalar.copy(ystack[:, (4 + m) * TN:(5 + m) * TN], yps[:])
                return ystack

            # tile-0 mix first so the PE has work as soon as the first
            # input block lands (phase A's silu chain would stall it)
            ystack0 = emit_mix(0)

            # ---- phase A: h = silu(x_scalar @ A1 + b1) for all tiles ------
            # (keeps the scalar engine on the silu table before the single
            #  switch to the sqrt table for the rest of the kernel)
            hsb_t = []
            for t in range(MACRO):
                hh = cp.tile([128, 2 * TN], f16, tag=f"hsb{t}")
                for p in range(2):
                    hps = pp.tile([128, TN], f32, tag="acc")
                    for q in range(2):
                        g = 2 * p + q
                        c0 = t * NGROUP * TN + g * TN
                        nc.tensor.matmul(hps[:], a1[:, q * 128:(q + 1) * 128],
                                         xs[:, c0:c0 + TN],
                                         start=(q == 0), stop=(q == 1))
                    nc.scalar.activation(hh[:, p * TN:(p + 1) * TN], hps[:],
                                         AF.Silu, bias=b1r[:, 0:1])
                hsb_t.append(hh)

            def emit_comps(fsb, fsb2, t):
                # contraction over channels with CG coefficients; software
                # pipelined one iteration late so the PE never waits on the
                # (DVE-late) F streams
                comps = pp.tile([24, TN], f32, tag="acc")
                for k in range(26):
                    mv = (fsb[:, k * TN:(k + 1) * TN] if k < 18 else
                          fsb2[:, (k - 18) * TN:(k - 17) * TN])
                    nc.tensor.matmul(comps[:], co[:, k * 24:(k + 1) * 24],
                                     mv, start=(k == 0), stop=(k == 25))
                csb = wp.tile([24, TN], f32, tag="csb")
                nc.scalar.copy(csb[:], comps[:])
                nc.sync.dma_start(OUT[t], csb[:])

            prev = None
            for t in range(MACRO):
                ystack = ystack0 if t == 0 else emit_mix(t)

                if prev is not None:
                    emit_comps(*prev)

                # ---- squares + per-l sums (DVE; Pool sharing a tile with
                # DVE stalls DVE reads ~10x, so Pool gets isolated work) ----
                sq = wp.tile([128, 9 * TN], f16, tag="sq")
                nc.vector.tensor_mul(sq[:], ystack[:], ystack[:])
                ssq = wp.tile([128, 2 * TN], f16, tag="ssq")
                tmp2 = wp.tile([128, 2 * TN], f16, tag="tmp2")
                # (y1[0]^2+y1[1]^2 | y2[0]^2+y2[1]^2)
                ia = sq[:, TN:9 * TN].rearrange("p (k n) -> p k n", k=8)
                nc.vector.tensor_add(
                    tmp2[:].rearrange("p (k n) -> p k n", k=2),
                    ia[:, 0:4:3, :], ia[:, 1:5:3, :])
                nc.vector.tensor_add(ssq[:, 0:TN], tmp2[:, 0:TN],
                                     sq[:, 3 * TN:4 * TN])
                t2 = wp.tile([128, TN], f16, tag="t2")
                nc.vector.tensor_add(t2[:], tmp2[:, TN:2 * TN],
                                     sq[:, 6 * TN:7 * TN])
                nc.vector.tensor_add(t2[:], t2[:], sq[:, 7 * TN:8 * TN])
                nc.vector.tensor_add(ssq[:, TN:2 * TN], t2[:],
                                     sq[:, 8 * TN:9 * TN])

                # ---- norm factors: pat_l = 1/(mean_l + eps), no sqrt ------
                rsum = pp.tile([96, TN], f32, tag="rb", bufs=2)
                for l, rhs in enumerate((sq[:, 0:TN], ssq[:, 0:TN],
                                         ssq[:, TN:2 * TN])):
                    nc.tensor.matmul(rsum[32 * l:32 * l + 32, :],
                                     on96[:, 32 * l:32 * l + 32], rhs,
                                     start=True, stop=True)
                msq = wp.tile([96, TN], f32, tag="msq")
                nc.scalar.activation(msq[:], rsum[:], AF.Copy, bias=1e-5)
                pat = wp.tile([96, TN], f32, tag="pat")
                nc.vector.reciprocal_approx_fast(pat[:], msq[:])
                # nodes are ones-padded on host, so pat is O(1) everywhere
                pat16 = wp.tile([96, TN], f16, tag="pat16")
                nc.vector.tensor_copy(pat16[:], pat[:])

                # broadcast patterns to (group, chan) partitions; cross term
                # 1/(rms0*rms2) = sqrt(pat0*pat2) computed post-broadcast
                bsb = wp.tile([128, 4 * TN], f16, tag="bsb")
                for l in range(3):
                    bps = pp.tile([128, TN], f32, tag="rb", bufs=2)
                    nc.tensor.matmul(bps[:], pbx[32 * l:32 * l + 4, :],
                                     pat16[32 * l:32 * l + 4, :],
                                     start=True, stop=True)
                    nc.scalar.copy(bsb[:, l * TN:(l + 1) * TN], bps[:])
                sqf3 = wp.tile([128, TN], f16, tag="sqf3")
                nc.vector.tensor_mul(sqf3[:], bsb[:, 0:TN],
                                     bsb[:, 2 * TN:3 * TN])
                nc.scalar.activation(bsb[:, 3 * TN:4 * TN], sqf3[:], AF.Sqrt)

                # ---- fold rms pairs into tp weights -----------------------
                # wsb order: g0 g1 g2 g56 g7 g8 ; pattern j -> 0 1 2 3 1 2
                wq = wp.tile([128, 6 * TN], f16, tag="wq")
                for j in range(6):
                    wps = pp.tile([128, TN], f32, tag="wps", bufs=2)
                    for pr in range(2):
                        nc.tensor.matmul(wps[:],
                                         a2[:, (2 * j + pr) * 128:
                                              (2 * j + pr + 1) * 128],
                                         hsb_t[t][:, pr * TN:(pr + 1) * TN],
                                         start=(pr == 0), stop=(pr == 1))
                    nc.scalar.copy(wq[:, j * TN:(j + 1) * TN], wps[:])
                wsb = wp.tile([128, 6 * TN], f16, tag="wsb")
                nc.vector.tensor_mul(wsb[:, 0:4 * TN], wq[:, 0:4 * TN],
                                     bsb[:, 0:4 * TN])
                nc.vector.tensor_mul(wsb[:, 4 * TN:6 * TN],
                                     wq[:, 4 * TN:6 * TN],
                                     bsb[:, TN:3 * TN])

                # ---- TP products into F streams ---------------------------
                # F0+F1+F2 share the c0 coef row, so they merge into ONE
                # contraction stream (saves 2 comps matmuls per tile)
                fsb = wp.tile([128, 18 * TN], f16, tag="fsb")
                f12 = wp.tile([128, 2 * TN], f16, tag="f12")
                nc.vector.tensor_mul(fsb[:, 0:TN], wsb[:, 0:TN], sq[:, 0:TN])
                nc.vector.tensor_mul(f12[:], wsb[:, TN:3 * TN],
                                     ssq[:, 0:2 * TN])
                nc.vector.tensor_add(fsb[:, 0:TN], fsb[:, 0:TN], f12[:, 0:TN])
                nc.vector.tensor_add(fsb[:, 0:TN], fsb[:, 0:TN],
                                     f12[:, TN:2 * TN])
                # i56: wy0 = g56'*y0 ; F[c] = wy0*y2[c]
                wy0 = wp.tile([128, TN], f16, tag="wy0")
                nc.vector.tensor_mul(wy0[:], wsb[:, 3 * TN:4 * TN],
                                     ystack[:, 0:TN])
                nc.vector.tensor_mul(
                    fsb[:, TN:6 * TN].rearrange("p (k n) -> p k n", k=5),
                    wy0[:].unsqueeze(1).broadcast_to((128, 5, TN)),
                    ystack[:, 4 * TN:9 * TN].rearrange("p (k n) -> p k n", k=5))
                # i7: wy1[a] = g7'*y1[a] ; F pairs by b
                wy1 = wp.tile([128, 3 * TN], f16, tag="wy1")
                nc.vector.tensor_mul(
                    wy1[:].rearrange("p (k n) -> p k n", k=3),
                    wsb[:, 4 * TN:5 * TN].unsqueeze(1).broadcast_to((128, 3, TN)),
                    ystack[:, TN:4 * TN].rearrange("p (k n) -> p k n", k=3))
                off = 6 * TN
                for b in range(3):
                    w_ = (b + 1)
                    nc.vector.tensor_mul(
                        fsb[:, off:off + w_ * TN].rearrange(
                            "p (k n) -> p k n", k=w_),
                        wy1[:, 0:w_ * TN].rearrange("p (k n) -> p k n", k=w_),
                        ystack[:, (1 + b) * TN:(2 + b) * TN]
                        .unsqueeze(1).broadcast_to((128, w_, TN)))
                    off += w_ * TN
                # i8: wy2[a] = g8'*y2[a] ; F pairs by b (skip (0,4));
                # b=0..2 on DVE into fsb, b=3..4 on Pool into its own tile
                wy2 = wp.tile([128, 5 * TN], f16, tag="wy2")
                nc.vector.tensor_mul(
                    wy2[:].rearrange("p (k n) -> p k n", k=5),
                    wsb[:, 5 * TN:6 * TN].unsqueeze(1).broadcast_to((128, 5, TN)),
                    ystack[:, 4 * TN:9 * TN].rearrange("p (k n) -> p k n", k=5))
                for b in range(3):
                    w_ = b + 1
                    nc.vector.tensor_mul(
                        fsb[:, off:off + w_ * TN].rearrange(
                            "p (k n) -> p k n", k=w_),
                        wy2[:, 0:w_ * TN].rearrange("p (k n) -> p k n", k=w_),
                        ystack[:, (4 + b) * TN:(5 + b) * TN]
                        .unsqueeze(1).broadcast_to((128, w_, TN)))
                    off += w_ * TN
                # last tile: its comps can't hide behind a next tile, so
                # start the contraction on the already-complete streams
                # before the final fsb2 products are issued
                last = (t == MACRO - 1)
                if last:
                    comps = pp.tile([24, TN], f32, tag="acc")
                    for k in range(18):
                        nc.tensor.matmul(comps[:], co[:, k * 24:(k + 1) * 24],
                                         fsb[:, k * TN:(k + 1) * TN],
                                         start=(k == 0), stop=False)

                # Pool tensor ops starve concurrent DVE ops of SBUF
                # bandwidth (~10x slowdown), so everything stays on DVE
                fsb2 = wp.tile([128, 8 * TN], f16, tag="fsb2")
                nc.vector.tensor_mul(
                    fsb2[:, 0:4 * TN].rearrange("p (k n) -> p k n", k=4),
                    wy2[:, 0:4 * TN].rearrange("p (k n) -> p k n", k=4),
                    ystack[:, 7 * TN:8 * TN]
                    .unsqueeze(1).broadcast_to((128, 4, TN)))
                nc.vector.tensor_mul(
                    fsb2[:, 4 * TN:8 * TN].rearrange("p (k n) -> p k n", k=4),
                    wy2[:, TN:5 * TN].rearrange("p (k n) -> p k n", k=4),
                    ystack[:, 8 * TN:9 * TN]
                    .unsqueeze(1).broadcast_to((128, 4, TN)))

                if last:
                    for k in range(18, 26):
                        nc.tensor.matmul(comps[:], co[:, k * 24:(k + 1) * 24],
                                         fsb2[:, (k - 18) * TN:(k - 17) * TN],
                                         start=False, stop=(k == 25))
                    csb = wp.tile([24, TN], f32, tag="csb")
                    nc.scalar.copy(csb[:], comps[:])
                    nc.sync.dma_start(OUT[t], csb[:])
                else:
                    prev = (fsb, fsb2, t)

    nc.compile()
    return nc


def _host_prep(inputs):
    xs = np.asarray(inputs["x_scalar"], dtype=np.float32)
    xq = np.asarray(inputs["x_spherical"], dtype=np.float32)
    W0 = np.asarray(inputs["W0"], np.float32)
    W1 = np.asarray(inputs["W1"], np.float32)
    W2 = np.asarray(inputs["W2"], np.float32)
    A1 = np.asarray(inputs["A1"], np.float32)
    b1 = np.asarray(inputs["b1"], np.float32)
    A2 = np.asarray(inputs["A2"], np.float32)
    p0 = np.asarray(inputs["p0"], np.float64)
    p2 = np.asarray(inputs["p2"], np.float64)

    NPAD = NCORES * NP
    xsp = np.ones((NPAD, 128), np.float32)
    xqp = np.ones((NPAD, 480), np.float32)
    for i in range(NCORES):
        s = slice(i * NSHARD, (i + 1) * NSHARD)
        d = slice(i * NP, i * NP + NSHARD)
        xsp[d] = xs[s]
        xqp[d] = xq[s]

    # per-core transposed shards (fp16, one packed tensor per macro tile)
    shards = []
    for i in range(NCORES):
        blk = xqp[i * NP:(i + 1) * NP]           # [NP, 480]
        x0t = blk[:, :128].T                     # [128, NP]
        x1t = blk[:, 128:320].reshape(NP, 64, 3).transpose(2, 1, 0)
        v1 = x1t.reshape(3, 64, MACRO, 2, 2, TN)        # m u t p q n
        x1t = v1.transpose(0, 2, 4, 1, 3, 5).reshape(3, MACRO, 128, 2 * TN)
        x2t = blk[:, 320:480].reshape(NP, 32, 5).transpose(2, 1, 0)
        v2 = x2t.reshape(5, 32, MACRO, 4, TN)           # m u t g n
        x2t = v2.transpose(0, 2, 3, 1, 4).reshape(5, MACRO, 128, TN)
        # xb[t] = [x0 (1600) | x1 m-major (2400) | x2 m-major (2000)]
        xb = np.empty((MACRO, 128, 6000), np.float16)
        for t in range(MACRO):
            xb[t, :, 0:1600] = x0t[:, t * 1600:(t + 1) * 1600]
            xb[t, :, 1600:4000] = (x1t[:, t].transpose(1, 0, 2)
                                   .reshape(128, 2400))
            xb[t, :, 4000:6000] = (x2t[:, t].transpose(1, 0, 2)
                                   .reshape(128, 2000))
        xst = np.ascontiguousarray(
            xsp[i * NP:(i + 1) * NP].T).astype(np.float16)
        shards.append((xst, np.ascontiguousarray(xb)))

    # folded constants
    alpha0 = 1.0 / sqrt(3 * HC)
    alpha2 = sqrt(5.0) / sqrt(4 * HC)
    cJ = [alpha0 * p0[0], _SGN110 * alpha0 * p0[1] / sqrt(3),
          alpha0 * p0[2] / sqrt(5)]
    cJ = [c / sqrt(3) for c in cJ]
    a2f = np.zeros((6, 64, 32), np.float64)
    a2f[0] = A2[:, 0:32] * cJ[0]
    a2f[1] = A2[:, 32:64] * cJ[1]
    a2f[2] = A2[:, 64:96] * cJ[2]
    a2f[3] = (alpha2 / (2 * sqrt(5))) * (p2[0] * A2[:, 160:192]
                                         + p2[1] * A2[:, 192:224])
    a2f[4] = A2[:, 224:256] * (alpha2 * p2[2] / 2.0)
    a2f[5] = A2[:, 256:288] * (alpha2 * p2[3] / 2.0)
    a2bd = np.zeros((6, 2, 128, 128), np.float32)
    for j in range(6):
        for pr in range(2):
            for q in range(2):
                g = 2 * pr + q
                a2bd[j, pr, 64 * q:64 * (q + 1), 32 * g:32 * (g + 1)] = a2f[j]
    a2bd = a2bd.reshape(12, 128, 128)

    w1bd = np.zeros((2, 128, 128), np.float32)
    for p in range(2):
        for q in range(2):
            g = 2 * p + q
            w1bd[p, 64 * q:64 * (q + 1), 32 * g:32 * (g + 1)] = W1 / sqrt(64)
    w2bd = np.zeros((128, 128), np.float32)
    for g in range(4):
        w2bd[32 * g:32 * (g + 1), 32 * g:32 * (g + 1)] = W2 / sqrt(32)

    a1bd = np.zeros((2, 128, 128), np.float32)
    for q in range(2):
        a1bd[q, :, 64 * q:64 * (q + 1)] = A1
    w0bd = np.zeros((4, 128, 128), np.float32)
    for g in range(4):
        w0bd[g, :, 32 * g:32 * (g + 1)] = W0 / sqrt(128)

    # [128, 96] selector: col 32l+g contracts group g scaled by 1/(HC(2l+1))
    on96 = np.zeros((128, 96), np.float32)
    for l in range(3):
        for g in range(4):
            on96[32 * g:32 * (g + 1), 32 * l + g] = 1.0 / (HC * (2 * l + 1))

    # [96, 128] broadcast selector, replicated at each 32l block so the
    # stationary/moving base partitions match (row 32l+g -> group g chans)
    pbx = np.zeros((96, 128), np.float32)
    for l in range(3):
        for g in range(4):
            pbx[32 * l + g, 32 * g:32 * (g + 1)] = 1.0
    c26 = np.concatenate([_COEF6[0:1], _COEF6[3:]], axis=0)   # F012 merged
    coef = np.zeros((26, 128, 24), np.float32)
    for k in range(26):
        for g in range(4):
            coef[k, 32 * g:32 * (g + 1), 6 * g:6 * (g + 1)] = c26[k]

    const = {
        "a1": a1bd.astype(np.float16),
        "w0": w0bd.astype(np.float16),
        "w1": w1bd.astype(np.float16),
        "w2": w2bd.astype(np.float16),
        "a2": a2bd.astype(np.float16),
        "on96": on96.astype(np.float16),
        "pbx": pbx.astype(np.float16),
        "coef": coef.astype(np.float16),
        "b1r": np.concatenate([b1, b1]).reshape(128, 1).astype(np.float32),
    }
    return shards, const


def kernel(**inputs):
    from concourse.bass_utils import run_bass_kernel_spmd

    if "nc" not in _NC_CACHE:
        _NC_CACHE["nc"] = _build_nc()
    nc = _NC_CACHE["nc"]

    shards, const = _host_prep(inputs)
    in_maps = []
    for i in range(NCORES):
        xst, xbt = shards[i]
        m = {"xs": xst, "xb": xbt}
        m.update(const)
        in_maps.append(m)

    res = run_bass_kernel_spmd(nc, in_maps, list(range(NCORES)))
    snode = np.concatenate(
        [res.results[i]["out"].reshape(MACRO, 4, 6, TN)
         .transpose(2, 0, 1, 3).reshape(6, NP)[:, :NSHARD]
         for i in range(NCORES)], axis=1)

    # sph (6 comps) -> cartesian 3x3, segment-sum, roll
    Q6 = np.concatenate([_QB[0].reshape(9, 1), _QB[2].reshape(9, 5)],
                        axis=1).astype(np.float32)     # [9, 6]
    cart = snode.T @ Q6.T                              # [N, 9]
    batch = np.asarray(inputs["batch"])
    B = int(inputs["num_graphs"])
    idx = np.searchsorted(batch, np.arange(B))
    g = np.add.reduceat(cart, idx, axis=0)
    g[np.diff(np.concatenate([idx, [N_FULL]])) == 0] = 0
    out = g.reshape(B, 3, 3).astype(np.float32)
    return np.roll(np.roll(out, 1, axis=1), 1, axis=2)
